# revision 46
# baseline (speedup 1.0000x reference)
"""MAGAT GNN message-passing kernel for 8 Trainium2 NeuronCores.

Math: the reference applies Sinkhorn-Knopp to adj0 but only uses the result
via `adj > 0`, and Sinkhorn preserves the zero/positive pattern exactly in
fp32. So attention is a masked softmax with mask = (adj0 > 0).

The mask is dense-complement sparse: for uniform-random adj0 only a handful
of entries are exactly zero (9 of 67M for the reference distribution). The
device therefore computes the DENSE (unmasked) attention — which needs no
adjacency traffic at all — and the host precomputes an exact per-row
correction tensor for the complement (the masked entries):

    h_num[i,:] = sum_j q[i,j] * [Wh_j | 1]  -  sum_{j masked} q[i,j] * [Wh_j | 1]
                 \------- device -------/      \------ host (corr) -------/

Softmax rows are scale-invariant, so exp(leaky_relu(e_src_i + e_dst_j))
reduces (dropping the per-row factor exp(e_src_i)) to

    q[i,j] = max(exp(-0.8*e_src_i) * exp(0.2*e_dst_j), exp(e_dst_j))
           = max(R_i * rB_j, B_j)

one elementwise op per [128j, 1024i] tile in bf16: DVE tensor_scalar
(mult, max) for most chunks, ACT relu(rB_j*R - B_j) for a subset (that
variant accumulates q - B_j; the missing sum_j B_j*[Wh_j|1] over those
chunks is folded into the host correction). The softmax denominator rides
along as a ones-column in the matmul rhs. This removes the 16MB/core
adjacency DMA and all per-element transcendentals: per chunk it is one
DVE/ACT op + 8 accumulating matmuls.

Sharding: 8 cores = 4 heads x 2 row-halves, as in the hint. x0 is rolled
per-core so own rows are device rows 0..2048 (identical SPMD program).
"""

import numpy as np
import ml_dtypes
from contextlib import ExitStack

import concourse.bacc as bacc
import concourse.mybir as mybir
import concourse.tile as tile
from concourse.bass_utils import run_bass_kernel_spmd

F32 = mybir.dt.float32
BF16 = mybir.dt.bfloat16
N, F, H, D = 4096, 128, 4, 128
NH = N // 2          # own rows per core
NC = N // 128        # 32 j-chunks
IPASS = 2            # i splits (PSUM: 8 banks of [128, 512] f32)
IW = NH // IPASS     # 1024 i per pass
BF = ml_dtypes.bfloat16
ACT_JC = frozenset(jc for jc in range(NC) if jc % 3 == 2 and 3 < jc < 30)

_cache = {}


def _build():
    nc = bacc.Bacc("TRN2", target_bir_lowering=False, debug=False)
    x0T = nc.dram_tensor("x0T", [F, N], BF16, kind="ExternalInput").ap()
    x0own = nc.dram_tensor("x0own", [NH, F], F32, kind="ExternalInput").ap()
    wta = nc.dram_tensor("wta", [D, F + 2], BF16, kind="ExternalInput").ap()
    w = nc.dram_tensor("w", [F, D], BF16, kind="ExternalInput").ap()
    corr = nc.dram_tensor("corr", [NH, D + 1], F32, kind="ExternalInput").ap()
    out = nc.dram_tensor("out", [NH, D], F32, kind="ExternalOutput").ap()

    with tile.TileContext(nc) as tc, ExitStack() as ctx:
        const = ctx.enter_context(tc.tile_pool(name="const", bufs=1))
        setup = ctx.enter_context(tc.tile_pool(name="setup", bufs=2))
        wpsum = ctx.enter_context(tc.tile_pool(name="wpsum", bufs=4, space="PSUM"))
        work = ctx.enter_context(tc.tile_pool(name="work", bufs=16))
        epil = ctx.enter_context(tc.tile_pool(name="epil", bufs=2))
        mpsum = ctx.enter_context(tc.tile_pool(name="mpsum", bufs=1, space="PSUM"))

        # persistent tiles
        x0T_sb = const.tile([128, N], BF16)            # [f, j] (j rolled)
        whp = const.tile([128, NC * (D + 2)], BF16)    # [Wh | 1 | pad] per chunk
        whp3 = whp[:].rearrange("p (c q) -> p c q", c=NC)
        Rb = const.tile([128, NH], BF16)               # exp(-0.8 e_src_i) bcast
        rB_sb = const.tile([128, NC], F32)             # exp(0.2 e_dst_j)
        B_sb = const.tile([128, NC], F32)              # exp(e_dst_j)
        nB_sb = const.tile([128, NC], F32)             # -exp(e_dst_j)
        x0o_sb = const.tile([128, IPASS * 8 * D], F32)
        corr_sb = const.tile([128, IPASS * 8 * (D + 1)], F32)
        corr4 = corr_sb[:].rearrange("p (q m e) -> p q m e", q=IPASS, m=8)

        # ~1us of dummy matmuls on zeroed tiles: fits inside the initial
        # DMA wait and releases the HAM clock throttle early
        nc.vector.memset(whp3[:, :, D], 1.0)   # ones column, no deps
        dml = setup.tile([128, 128], BF16, name="dml")
        nc.vector.memset(dml[:], 0.0)
        dmr = setup.tile([128, 512], BF16, name="dmr")
        nc.vector.memset(dmr[:], 0.0)
        for u in range(2):
            dps = wpsum.tile([128, 512], F32, tag="psw", name=f"dps{u}")
            nc.tensor.matmul(dps[:], lhsT=dml[:], rhs=dmr[:],
                             start=True, stop=True)

        # smallest inputs first ([wT | a2] unblocks the whole setup chain),
        # then x0T pieces, then epilogue-only data
        wta_sb = setup.tile([D, F + 2], BF16)
        nc.scalar.dma_start(wta_sb[:], wta)   # ACT HWDGE: parallel with sync queue
        wT_sb = wta_sb[:, 0:F]
        a2_sb = wta_sb[:, F:F + 2]
        w_sb = setup.tile([F, D], BF16)
        nc.sync.dma_start(w_sb[:], w)
        for lo, hi in ((0, 512), (512, 1024), (1024, 2048), (2048, 3072),
                       (3072, 4096)):
            nc.sync.dma_start(x0T_sb[:, lo:hi], x0T[:, lo:hi])
        nc.sync.dma_start(
            x0o_sb[:].rearrange("p (q m d) -> p q m d", q=IPASS, m=8),
            x0own.rearrange("(q m p) d -> p q m d", q=IPASS, p=128))
        nc.sync.dma_start(
            corr4[:, :, :, :],
            corr.rearrange("(q m p) e -> p q m e", q=IPASS, p=128))

        # [ua | ub] = W @ [a_src | a_dst]  ([f] vectors)
        psu = wpsum.tile([128, 2], F32, tag="psw", name="psu")
        nc.tensor.matmul(psu[:], lhsT=wT_sb, rhs=a2_sb[:],
                         start=True, stop=True)
        ua_b = setup.tile([128, 128], BF16)        # ua bcast along free
        nc.vector.tensor_copy(ua_b[:], psu[:, 0:1].broadcast_to([128, 128]))
        wub = setup.tile([128, D + 1], BF16)       # [W | ub] matmul rhs
        nc.vector.tensor_copy(wub[:, 0:D], w_sb[:])
        nc.vector.tensor_copy(wub[:, D:D + 1], psu[:, 1:2])

        # esb[p, i] = e_src_i (broadcast over partitions), own rows only;
        # g2/g3 feed only pass 1 and are emitted after the steady ramp
        def emit_esb(g):
            esb = wpsum.tile([128, 512], F32, tag="psw", name=f"esb{g}")
            nc.tensor.matmul(esb[:], lhsT=ua_b[:],
                             rhs=x0T_sb[:, g * 512:(g + 1) * 512],
                             start=True, stop=True)
            nc.scalar.activation(Rb[:, g * 512:(g + 1) * 512], esb[:],
                                 mybir.ActivationFunctionType.Exp, scale=-0.8)

        for g in range(2):
            emit_esb(g)

        # [Wh | e_dst] per chunk in one matmul (rhs = [W | ub]); groups of 3
        # chunks per PSUM bank; group emission is interleaved with the first
        # steady chunks so neither engine queue serializes the ramp
        def emit_group(g):
            k0 = 3 * g
            nk = min(3, NC - k0)
            psw = wpsum.tile([128, nk * (D + 1)], F32, tag="psw",
                             name=f"psw{g}")
            psw3 = psw[:].rearrange("p (k q) -> p k q", k=nk)
            for k in range(nk):
                cs = slice((k0 + k) * 128, (k0 + k + 1) * 128)
                nc.tensor.matmul(psw3[:, k, :], lhsT=x0T_sb[:, cs],
                                 rhs=wub[:], start=True, stop=True)
            gs = slice(k0, k0 + nk)
            nc.vector.tensor_copy(whp3[:, gs, 0:D], psw3[:, :, 0:D])
            nc.scalar.activation(rB_sb[:, gs], psw3[:, :, D],
                                 mybir.ActivationFunctionType.Exp, scale=0.2)
            nc.scalar.activation(B_sb[:, gs], psw3[:, :, D],
                                 mybir.ActivationFunctionType.Exp)
            if any(jc in ACT_JC for jc in range(k0, k0 + nk)):
                nc.vector.tensor_scalar_mul(nB_sb[:, gs], B_sb[:, gs], -1.0)

        for g in range(4):
            emit_group(g)

        # steady state: the 8 [128,129] accumulators are packed at stride
        # 256 f32 (2 per PSUM bank, each within a bank) -> 4 banks total
        def emit_chunk(ip, jc, pssm):
            iw = slice(ip * IW, (ip + 1) * IW)
            q = work.tile([128, IW], BF16, tag="q")
            if jc in ACT_JC:
                # accumulates q - B_j; the missing sum_j B_j*whp_j is
                # pre-folded into corr on the host
                nc.scalar.activation(q[:], Rb[:, iw],
                                     mybir.ActivationFunctionType.Relu,
                                     bias=nB_sb[:, jc:jc + 1],
                                     scale=rB_sb[:, jc:jc + 1])
            else:
                nc.vector.tensor_scalar(
                    out=q[:], in0=Rb[:, iw], scalar1=rB_sb[:, jc:jc + 1],
                    scalar2=B_sb[:, jc:jc + 1],
                    op0=mybir.AluOpType.mult, op1=mybir.AluOpType.max)
            for m in range(8):
                # accumulators sit 2-per-bank at 1KB stride: start=True
                # clears the whole bank, so only the even-m matmul sets it;
                # the odd-m region then starts from has_written=0
                nc.tensor.matmul(pssm(m),
                                 lhsT=q[:, m * 128:(m + 1) * 128],
                                 rhs=whp3[:, jc, 0:D + 1],
                                 start=(jc == 0 and m % 2 == 0),
                                 stop=(jc == NC - 1))

        def emit_epilogue(ip, pssb):
            # epilogue: h = acc - corr; y = elu(elu(h/den) + x0).
            # PSUM evacuation in quarters (small pass-boundary stall), the
            # rest of the chain in halves to pipeline DVE/ACT
            hs = epil.tile([128, 8 * (D + 1)], F32, tag="hs", name=f"hs_{ip}")
            hs3 = hs[:].rearrange("p (m e) -> p m e", m=8)
            # tiny den-only subtract first so the reciprocal overlaps the
            # big numerator subtract instead of waiting behind it
            den8 = epil.tile([128, 8], F32, tag="den8", name=f"den8_{ip}")
            nc.vector.tensor_sub(den8[:], pssb(None)[:, :, D],
                                 corr4[:, ip, :, D])
            rec8 = epil.tile([128, 8], F32, tag="rec8", name=f"rec8_{ip}")
            nc.vector.reciprocal(rec8[:], den8[:])
            for hl in range(2):
                ms = slice(hl * 4, (hl + 1) * 4)
                tg = f"{ip}_{hl}"
                nc.vector.tensor_sub(hs3[:, ms, 0:D], pssb(None)[:, ms, 0:D],
                                     corr4[:, ip, ms, 0:D])
                rec = rec8[:, ms]
                hpn = epil.tile([128, 4 * D], F32, tag=f"hpn{hl}", name=f"hpn_{tg}")
                hpn3 = hpn[:].rearrange("p (m d) -> p m d", m=4)
                nc.vector.tensor_mul(hpn3[:, :, :], hs3[:, ms, 0:D],
                                     rec[:, :, None].broadcast_to([128, 4, D]))
                # elu(x) = max(x, exp(min(x,0)) - 1)
                t1 = epil.tile([128, 4 * D], F32, tag=f"t1{hl}", name=f"t1_{tg}")
                nc.vector.tensor_scalar_min(t1[:], hpn[:], 0.0)
                ex1 = epil.tile([128, 4 * D], F32, tag=f"ex1{hl}", name=f"ex1_{tg}")
                nc.scalar.activation(ex1[:], t1[:], mybir.ActivationFunctionType.Exp)
                el1 = epil.tile([128, 4 * D], F32, tag=f"el1{hl}", name=f"el1_{tg}")
                nc.vector.scalar_tensor_tensor(
                    out=el1[:], in0=ex1[:], scalar=-1.0, in1=hpn[:],
                    op0=mybir.AluOpType.add, op1=mybir.AluOpType.max)
                # residual + second elu
                r8 = epil.tile([128, 4 * D], F32, tag=f"r8{hl}", name=f"r8_{tg}")
                nc.vector.tensor_add(
                    r8[:], el1[:],
                    x0o_sb[:, (ip * 8 + hl * 4) * D:(ip * 8 + hl * 4 + 4) * D])
                t2 = epil.tile([128, 4 * D], F32, tag=f"t2{hl}", name=f"t2_{tg}")
                nc.vector.tensor_scalar_min(t2[:], r8[:], 0.0)
                ex2 = epil.tile([128, 4 * D], F32, tag=f"ex2{hl}", name=f"ex2_{tg}")
                nc.scalar.activation(ex2[:], t2[:], mybir.ActivationFunctionType.Exp)
                y8 = epil.tile([128, 4 * D], F32, tag=f"y8{hl}", name=f"y8_{tg}")
                nc.vector.scalar_tensor_tensor(
                    out=y8[:], in0=ex2[:], scalar=-1.0, in1=r8[:],
                    op0=mybir.AluOpType.add, op1=mybir.AluOpType.max)
                y83 = y8[:].rearrange("p (m d) -> p m d", m=4)
                ov = out.rearrange("(q l m p) d -> q l p m d",
                                   q=IPASS, l=2, p=128)
                eng = nc.sync if hl == 0 else nc.scalar
                eng.dma_start(ov[ip][hl], y83[:, :, :])

        def slicers(pss):
            def pssm(m):
                return pss[:, m * 256:m * 256 + D + 1]
            def pssb(b):
                return pss[:].rearrange("p (m k) -> p m k", k=256)
            return pssm, pssb

        pssA = mpsum.tile([128, 8 * 256], F32, tag="acc", name="acc_0")
        pmA, pbA = slicers(pssA)
        for jc in range(8):
            emit_chunk(0, jc, pmA)
        emit_esb(2)
        emit_esb(3)
        for g in range(4, 11):
            emit_group(g)
        for jc in range(8, NC):
            emit_chunk(0, jc, pmA)
        pssB = mpsum.tile([128, 8 * 256], F32, tag="acc", name="acc_1")
        pmB, pbB = slicers(pssB)
        # emit the next pass's first chunks before the previous epilogue so
        # the q tiles are ready the moment the accumulator banks drain
        for jc in range(4):
            emit_chunk(1, jc, pmB)
        emit_epilogue(0, pbA)
        for jc in range(4, NC):
            emit_chunk(1, jc, pmB)
        emit_epilogue(1, pbB)

    nc.compile()
    return nc


def _get_nc():
    if "nc" not in _cache:
        _cache["nc"] = _build()
    return _cache["nc"]


def _make_in_maps(x0, adj0, W, a_src, a_dst):
    """Host prep: per-core layout transforms + the correction tensor
    (exact masked-entry complement, plus the B*whp fold for ACT chunks)."""
    x0 = np.asarray(x0, np.float32)
    W = np.asarray(W, np.float32)
    a_src = np.asarray(a_src, np.float32)
    a_dst = np.asarray(a_dst, np.float32)

    corr = [np.zeros((NH, D + 1), np.float32) for _ in range(8)]
    zh, zi, zj = np.nonzero(np.asarray(adj0) == 0.0)
    act_j = np.zeros(N, np.bool_)            # device j-rows handled on ACT
    for jc in ACT_JC:
        act_j[jc * 128:(jc + 1) * 128] = True

    for h in range(H):
        Wh = x0 @ W[h]                                   # [N, D]
        es = Wh @ a_src[h]                               # [N]
        ed = Wh @ a_dst[h]                               # [N]
        B = np.exp(ed)
        whp1 = np.concatenate([Wh, np.ones((N, 1), np.float32)], 1)  # [N, 129]
        for half in range(2):
            c = h * 2 + half
            i0 = half * NH
            # ACT chunks accumulate q - B_j: fold sum_j B_j*whp_j back in.
            # Device j-row k holds global j = (k + i0) % N.
            jg = (np.arange(N) + i0) % N
            sel = jg[act_j]
            corr[c] -= (B[sel][:, None] * whp1[sel]).sum(0)[None, :]
        # exact mask corrections for this head's zero entries
        sel = zh == h
        ii, jj = zi[sel], zj[sel]
        if ii.size:
            qv = np.maximum(np.exp(-0.8 * es[ii] + 0.2 * ed[jj]), B[jj])
            for t in range(ii.size):
                c = h * 2 + (0 if ii[t] < NH else 1)
                il = int(ii[t]) % NH
                corr[c][il] += qv[t] * whp1[jj[t]]

    in_maps = []
    for c in range(8):
        h, half = c // 2, c % 2
        i0 = half * NH
        xr = np.roll(x0, -i0, axis=0) if i0 else x0
        in_maps.append(dict(
            x0T=np.ascontiguousarray(xr.T).astype(BF),
            x0own=np.ascontiguousarray(x0[i0:i0 + NH]),
            wta=np.ascontiguousarray(np.concatenate(
                [W[h].T, a_src[h][:, None], a_dst[h][:, None]], 1)).astype(BF),
            w=np.ascontiguousarray(W[h]).astype(BF),
            corr=corr[c],
        ))
    return in_maps


def kernel(x0, adj0, W, a_src, a_dst):
    nc = _get_nc()
    in_maps = _make_in_maps(x0, adj0, W, a_src, a_dst)
    res = run_bass_kernel_spmd(nc, in_maps, core_ids=list(range(8))).results
    x1 = np.empty((N, H * D), np.float32)
    for c in range(8):
        h, half = c // 2, c % 2
        i0 = half * NH
        x1[i0:i0 + NH, h * D:(h + 1) * D] = res[c]["out"]
    return x1


# revision 48
# speedup vs baseline: 1.0206x; 1.0206x over previous
"""MAGAT GNN message-passing kernel for 8 Trainium2 NeuronCores.

Math: the reference applies Sinkhorn-Knopp to adj0 but only uses the result
via `adj > 0`, and Sinkhorn preserves the zero/positive pattern exactly in
fp32. So attention is a masked softmax with mask = (adj0 > 0).

The mask is dense-complement sparse: for uniform-random adj0 only a handful
of entries are exactly zero (9 of 67M for the reference distribution). The
device therefore computes the DENSE (unmasked) attention — which needs no
adjacency traffic at all — and the host precomputes an exact per-row
correction tensor for the complement (the masked entries):

    h_num[i,:] = sum_j q[i,j] * [Wh_j | 1]  -  sum_{j masked} q[i,j] * [Wh_j | 1]
                 \------- device -------/      \------ host (corr) -------/

Softmax rows are scale-invariant, so exp(leaky_relu(e_src_i + e_dst_j))
reduces (dropping the per-row factor exp(e_src_i)) to

    q[i,j] = max(exp(-0.8*e_src_i) * exp(0.2*e_dst_j), exp(e_dst_j))
           = max(R_i * rB_j, B_j)

one elementwise op per [128j, 1024i] tile in bf16: DVE tensor_scalar
(mult, max) for most chunks, ACT relu(rB_j*R - B_j) for a subset (that
variant accumulates q - B_j; the missing sum_j B_j*[Wh_j|1] over those
chunks is folded into the host correction). The softmax denominator rides
along as a ones-column in the matmul rhs. This removes the 16MB/core
adjacency DMA and all per-element transcendentals: per chunk it is one
DVE/ACT op + 8 accumulating matmuls.

Sharding: 8 cores = 4 heads x 2 row-halves, as in the hint. x0 is rolled
per-core so own rows are device rows 0..2048 (identical SPMD program).
"""

import numpy as np
import ml_dtypes
from contextlib import ExitStack

import concourse.bacc as bacc
import concourse.mybir as mybir
import concourse.tile as tile
from concourse.bass_utils import run_bass_kernel_spmd

F32 = mybir.dt.float32
BF16 = mybir.dt.bfloat16
N, F, H, D = 4096, 128, 4, 128
NH = N // 2          # own rows per core
NC = N // 128        # 32 j-chunks
IPASS = 2            # i splits (PSUM: 8 banks of [128, 512] f32)
IW = NH // IPASS     # 1024 i per pass
BF = ml_dtypes.bfloat16
ACT_JC = frozenset(jc for jc in range(NC) if jc % 3 == 2 and 3 < jc < 30)

_cache = {}


def _build():
    nc = bacc.Bacc("TRN2", target_bir_lowering=False, debug=False)
    x0T = nc.dram_tensor("x0T", [F, N], BF16, kind="ExternalInput").ap()
    x0own = nc.dram_tensor("x0own", [NH, F], F32, kind="ExternalInput").ap()
    wta = nc.dram_tensor("wta", [D, F + 2], BF16, kind="ExternalInput").ap()
    w = nc.dram_tensor("w", [F, D], BF16, kind="ExternalInput").ap()
    corr = nc.dram_tensor("corr", [NH, D + 1], F32, kind="ExternalInput").ap()
    out = nc.dram_tensor("out", [NH, D], F32, kind="ExternalOutput").ap()

    with tile.TileContext(nc) as tc, ExitStack() as ctx:
        const = ctx.enter_context(tc.tile_pool(name="const", bufs=1))
        setup = ctx.enter_context(tc.tile_pool(name="setup", bufs=2))
        wpsum = ctx.enter_context(tc.tile_pool(name="wpsum", bufs=4, space="PSUM"))
        work = ctx.enter_context(tc.tile_pool(name="work", bufs=16))
        epil = ctx.enter_context(tc.tile_pool(name="epil", bufs=2))
        mpsum = ctx.enter_context(tc.tile_pool(name="mpsum", bufs=1, space="PSUM"))

        # persistent tiles
        x0T_sb = const.tile([128, N], BF16)            # [f, j] (j rolled)
        whp = const.tile([128, NC * (D + 2)], BF16)    # [Wh | 1 | pad] per chunk
        whp3 = whp[:].rearrange("p (c q) -> p c q", c=NC)
        Rb = const.tile([128, NH], BF16)               # exp(-0.8 e_src_i) bcast
        rB_sb = const.tile([128, NC], F32)             # exp(0.2 e_dst_j)
        B_sb = const.tile([128, NC], F32)              # exp(e_dst_j)
        nB_sb = const.tile([128, NC], F32)             # -exp(e_dst_j)
        x0o_sb = const.tile([128, IPASS * 8 * D], F32)
        corr_sb = const.tile([128, IPASS * 8 * (D + 1)], F32)
        corr4 = corr_sb[:].rearrange("p (q m e) -> p q m e", q=IPASS, m=8)

        # ~1us of dummy matmuls on zeroed tiles: fits inside the initial
        # DMA wait and releases the HAM clock throttle early
        nc.vector.memset(whp3[:, :, D], 1.0)   # ones column, no deps
        dml = setup.tile([128, 128], BF16, name="dml")
        nc.vector.memset(dml[:], 0.0)
        dmr = setup.tile([128, 512], BF16, name="dmr")
        nc.vector.memset(dmr[:], 0.0)
        for u in range(2):
            dps = wpsum.tile([128, 512], F32, tag="psw", name=f"dps{u}")
            nc.tensor.matmul(dps[:], lhsT=dml[:], rhs=dmr[:],
                             start=True, stop=True)

        # smallest inputs first ([wT | a2] unblocks the whole setup chain),
        # then x0T pieces, then epilogue-only data
        wta_sb = setup.tile([D, F + 2], BF16)
        nc.sync.dma_start(wta_sb[:], wta)
        wT_sb = wta_sb[:, 0:F]
        a2_sb = wta_sb[:, F:F + 2]
        w_sb = setup.tile([F, D], BF16)
        nc.sync.dma_start(w_sb[:], w)
        for lo, hi in ((0, 512), (512, 1024), (1024, 2048), (2048, 3072),
                       (3072, 4096)):
            nc.sync.dma_start(x0T_sb[:, lo:hi], x0T[:, lo:hi])
        nc.sync.dma_start(
            x0o_sb[:].rearrange("p (q m d) -> p q m d", q=IPASS, m=8),
            x0own.rearrange("(q m p) d -> p q m d", q=IPASS, p=128))
        nc.sync.dma_start(
            corr4[:, :, :, :],
            corr.rearrange("(q m p) e -> p q m e", q=IPASS, p=128))

        # [ua | ub] = W @ [a_src | a_dst]  ([f] vectors)
        psu = wpsum.tile([128, 2], F32, tag="psw", name="psu")
        nc.tensor.matmul(psu[:], lhsT=wT_sb, rhs=a2_sb[:],
                         start=True, stop=True)
        ua_b = setup.tile([128, 128], BF16)        # ua bcast along free
        nc.vector.tensor_copy(ua_b[:], psu[:, 0:1].broadcast_to([128, 128]))
        wub = setup.tile([128, D + 1], BF16)       # [W | ub] matmul rhs
        nc.vector.tensor_copy(wub[:, 0:D], w_sb[:])
        nc.vector.tensor_copy(wub[:, D:D + 1], psu[:, 1:2])

        # esb[p, i] = e_src_i (broadcast over partitions), own rows only;
        # g2/g3 feed only pass 1 and are emitted after the steady ramp
        def emit_esb(g):
            esb = wpsum.tile([128, 512], F32, tag="psw", name=f"esb{g}")
            nc.tensor.matmul(esb[:], lhsT=ua_b[:],
                             rhs=x0T_sb[:, g * 512:(g + 1) * 512],
                             start=True, stop=True)
            nc.scalar.activation(Rb[:, g * 512:(g + 1) * 512], esb[:],
                                 mybir.ActivationFunctionType.Exp, scale=-0.8)

        for g in range(2):
            emit_esb(g)

        # [Wh | e_dst] per chunk in one matmul (rhs = [W | ub]); groups of 3
        # chunks per PSUM bank; group emission is interleaved with the first
        # steady chunks so neither engine queue serializes the ramp
        def emit_group(g):
            k0 = 3 * g
            nk = min(3, NC - k0)
            psw = wpsum.tile([128, nk * (D + 1)], F32, tag="psw",
                             name=f"psw{g}")
            psw3 = psw[:].rearrange("p (k q) -> p k q", k=nk)
            for k in range(nk):
                cs = slice((k0 + k) * 128, (k0 + k + 1) * 128)
                nc.tensor.matmul(psw3[:, k, :], lhsT=x0T_sb[:, cs],
                                 rhs=wub[:], start=True, stop=True)
            gs = slice(k0, k0 + nk)
            nc.vector.tensor_copy(whp3[:, gs, 0:D], psw3[:, :, 0:D])
            nc.scalar.activation(rB_sb[:, gs], psw3[:, :, D],
                                 mybir.ActivationFunctionType.Exp, scale=0.2)
            nc.scalar.activation(B_sb[:, gs], psw3[:, :, D],
                                 mybir.ActivationFunctionType.Exp)
            if any(jc in ACT_JC for jc in range(k0, k0 + nk)):
                nc.vector.tensor_scalar_mul(nB_sb[:, gs], B_sb[:, gs], -1.0)

        for g in range(4):
            emit_group(g)

        # steady state: the 8 [128,129] accumulators are packed at stride
        # 256 f32 (2 per PSUM bank, each within a bank) -> 4 banks total
        def emit_chunk(ip, jc, pssm):
            iw = slice(ip * IW, (ip + 1) * IW)
            q = work.tile([128, IW], BF16, tag="q")
            if jc in ACT_JC:
                # accumulates q - B_j; the missing sum_j B_j*whp_j is
                # pre-folded into corr on the host
                nc.scalar.activation(q[:], Rb[:, iw],
                                     mybir.ActivationFunctionType.Relu,
                                     bias=nB_sb[:, jc:jc + 1],
                                     scale=rB_sb[:, jc:jc + 1])
            else:
                nc.vector.tensor_scalar(
                    out=q[:], in0=Rb[:, iw], scalar1=rB_sb[:, jc:jc + 1],
                    scalar2=B_sb[:, jc:jc + 1],
                    op0=mybir.AluOpType.mult, op1=mybir.AluOpType.max)
            for m in range(8):
                # accumulators sit 2-per-bank at 1KB stride: start=True
                # clears the whole bank, so only the even-m matmul sets it;
                # the odd-m region then starts from has_written=0
                nc.tensor.matmul(pssm(m),
                                 lhsT=q[:, m * 128:(m + 1) * 128],
                                 rhs=whp3[:, jc, 0:D + 1],
                                 start=(jc == 0 and m % 2 == 0),
                                 stop=(jc == NC - 1))

        def emit_epilogue(ip, pssb):
            # epilogue: h = acc - corr; y = elu(elu(h/den) + x0).
            # PSUM evacuation in quarters (small pass-boundary stall), the
            # rest of the chain in halves to pipeline DVE/ACT
            hs = epil.tile([128, 8 * (D + 1)], F32, tag="hs", name=f"hs_{ip}")
            hs3 = hs[:].rearrange("p (m e) -> p m e", m=8)
            # tiny den-only subtract first so the reciprocal overlaps the
            # big numerator subtract instead of waiting behind it
            den8 = epil.tile([128, 8], F32, tag="den8", name=f"den8_{ip}")
            nc.vector.tensor_sub(den8[:], pssb(None)[:, :, D],
                                 corr4[:, ip, :, D])
            rec8 = epil.tile([128, 8], F32, tag="rec8", name=f"rec8_{ip}")
            nc.vector.reciprocal(rec8[:], den8[:])
            if ip == 0:
                # fast PSUM evacuation: one subtract, so the next pass's
                # matmuls stop WAR-waiting on this pass's readers sooner
                nc.vector.tensor_sub(hs3[:, :, 0:D], pssb(None)[:, :, 0:D],
                                     corr4[:, ip, :, 0:D])
            for hl in range(2):
                ms = slice(hl * 4, (hl + 1) * 4)
                tg = f"{ip}_{hl}"
                if ip > 0:
                    nc.vector.tensor_sub(hs3[:, ms, 0:D], pssb(None)[:, ms, 0:D],
                                         corr4[:, ip, ms, 0:D])
                rec = rec8[:, ms]
                hpn = epil.tile([128, 4 * D], F32, tag=f"hpn{hl}", name=f"hpn_{tg}")
                hpn3 = hpn[:].rearrange("p (m d) -> p m d", m=4)
                nc.vector.tensor_mul(hpn3[:, :, :], hs3[:, ms, 0:D],
                                     rec[:, :, None].broadcast_to([128, 4, D]))
                # elu(x) = max(x, exp(min(x,0)) - 1)
                t1 = epil.tile([128, 4 * D], F32, tag=f"t1{hl}", name=f"t1_{tg}")
                nc.vector.tensor_scalar_min(t1[:], hpn[:], 0.0)
                ex1 = epil.tile([128, 4 * D], F32, tag=f"ex1{hl}", name=f"ex1_{tg}")
                nc.scalar.activation(ex1[:], t1[:], mybir.ActivationFunctionType.Exp)
                el1 = epil.tile([128, 4 * D], F32, tag=f"el1{hl}", name=f"el1_{tg}")
                nc.vector.scalar_tensor_tensor(
                    out=el1[:], in0=ex1[:], scalar=-1.0, in1=hpn[:],
                    op0=mybir.AluOpType.add, op1=mybir.AluOpType.max)
                # residual + second elu
                r8 = epil.tile([128, 4 * D], F32, tag=f"r8{hl}", name=f"r8_{tg}")
                nc.vector.tensor_add(
                    r8[:], el1[:],
                    x0o_sb[:, (ip * 8 + hl * 4) * D:(ip * 8 + hl * 4 + 4) * D])
                t2 = epil.tile([128, 4 * D], F32, tag=f"t2{hl}", name=f"t2_{tg}")
                nc.vector.tensor_scalar_min(t2[:], r8[:], 0.0)
                ex2 = epil.tile([128, 4 * D], F32, tag=f"ex2{hl}", name=f"ex2_{tg}")
                nc.scalar.activation(ex2[:], t2[:], mybir.ActivationFunctionType.Exp)
                y8 = epil.tile([128, 4 * D], F32, tag=f"y8{hl}", name=f"y8_{tg}")
                nc.vector.scalar_tensor_tensor(
                    out=y8[:], in0=ex2[:], scalar=-1.0, in1=r8[:],
                    op0=mybir.AluOpType.add, op1=mybir.AluOpType.max)
                y83 = y8[:].rearrange("p (m d) -> p m d", m=4)
                ov = out.rearrange("(q l m p) d -> q l p m d",
                                   q=IPASS, l=2, p=128)
                eng = nc.sync if hl == 0 else nc.scalar
                eng.dma_start(ov[ip][hl], y83[:, :, :])

        def slicers(pss):
            def pssm(m):
                return pss[:, m * 256:m * 256 + D + 1]
            def pssb(b):
                return pss[:].rearrange("p (m k) -> p m k", k=256)
            return pssm, pssb

        pssA = mpsum.tile([128, 8 * 256], F32, tag="acc", name="acc_0")
        pmA, pbA = slicers(pssA)
        for jc in range(8):
            emit_chunk(0, jc, pmA)
        emit_esb(2)
        emit_esb(3)
        for g in range(4, 11):
            emit_group(g)
        for jc in range(8, NC):
            emit_chunk(0, jc, pmA)
        pssB = mpsum.tile([128, 8 * 256], F32, tag="acc", name="acc_1")
        pmB, pbB = slicers(pssB)
        # emit the next pass's first chunks before the previous epilogue so
        # the q tiles are ready the moment the accumulator banks drain
        for jc in range(4):
            emit_chunk(1, jc, pmB)
        emit_epilogue(0, pbA)
        for jc in range(4, NC):
            emit_chunk(1, jc, pmB)
        emit_epilogue(1, pbB)

    nc.compile()
    return nc


def _get_nc():
    if "nc" not in _cache:
        _cache["nc"] = _build()
    return _cache["nc"]


def _make_in_maps(x0, adj0, W, a_src, a_dst):
    """Host prep: per-core layout transforms + the correction tensor
    (exact masked-entry complement, plus the B*whp fold for ACT chunks)."""
    x0 = np.asarray(x0, np.float32)
    W = np.asarray(W, np.float32)
    a_src = np.asarray(a_src, np.float32)
    a_dst = np.asarray(a_dst, np.float32)

    corr = [np.zeros((NH, D + 1), np.float32) for _ in range(8)]
    zh, zi, zj = np.nonzero(np.asarray(adj0) == 0.0)
    act_j = np.zeros(N, np.bool_)            # device j-rows handled on ACT
    for jc in ACT_JC:
        act_j[jc * 128:(jc + 1) * 128] = True

    for h in range(H):
        Wh = x0 @ W[h]                                   # [N, D]
        es = Wh @ a_src[h]                               # [N]
        ed = Wh @ a_dst[h]                               # [N]
        B = np.exp(ed)
        whp1 = np.concatenate([Wh, np.ones((N, 1), np.float32)], 1)  # [N, 129]
        for half in range(2):
            c = h * 2 + half
            i0 = half * NH
            # ACT chunks accumulate q - B_j: fold sum_j B_j*whp_j back in.
            # Device j-row k holds global j = (k + i0) % N.
            jg = (np.arange(N) + i0) % N
            sel = jg[act_j]
            corr[c] -= (B[sel][:, None] * whp1[sel]).sum(0)[None, :]
        # exact mask corrections for this head's zero entries
        sel = zh == h
        ii, jj = zi[sel], zj[sel]
        if ii.size:
            qv = np.maximum(np.exp(-0.8 * es[ii] + 0.2 * ed[jj]), B[jj])
            for t in range(ii.size):
                c = h * 2 + (0 if ii[t] < NH else 1)
                il = int(ii[t]) % NH
                corr[c][il] += qv[t] * whp1[jj[t]]

    in_maps = []
    for c in range(8):
        h, half = c // 2, c % 2
        i0 = half * NH
        xr = np.roll(x0, -i0, axis=0) if i0 else x0
        in_maps.append(dict(
            x0T=np.ascontiguousarray(xr.T).astype(BF),
            x0own=np.ascontiguousarray(x0[i0:i0 + NH]),
            wta=np.ascontiguousarray(np.concatenate(
                [W[h].T, a_src[h][:, None], a_dst[h][:, None]], 1)).astype(BF),
            w=np.ascontiguousarray(W[h]).astype(BF),
            corr=corr[c],
        ))
    return in_maps


def kernel(x0, adj0, W, a_src, a_dst):
    nc = _get_nc()
    in_maps = _make_in_maps(x0, adj0, W, a_src, a_dst)
    res = run_bass_kernel_spmd(nc, in_maps, core_ids=list(range(8))).results
    x1 = np.empty((N, H * D), np.float32)
    for c in range(8):
        h, half = c // 2, c % 2
        i0 = half * NH
        x1[i0:i0 + NH, h * D:(h + 1) * D] = res[c]["out"]
    return x1
